# revision 84
# baseline (speedup 1.0000x reference)
"""DiffJPEG Trainium2 Bass kernel.

Strategy (pure data-parallel over batch, 4 images per core on 8 cores):
  - load RGB in natural row layout [128 rows, 3x512] (x = row%8 fully on
    partitions); image 0's first tile DMA is split into two w-halves so
    the first color STT starts ~1us earlier
  - RGB->Y via 2 fused scalar_tensor_tensor (Horner) on DVE
  - the JPEG -128/+128 shift is OMITTED on both sides: an integer shift
    of the DC coefficient by exactly -160 quant steps commutes with RNE
    rounding (round(x-160) == round(x)-160 and the diff_round cubic is
    shift-invariant), so forward-shift and inverse-add-back cancel
    bit-exactly and no ACT bias vectors are needed anywhere
  - stage-1 Y is DATA-STATIONARY (yt chunks as lhsT, block-diag DCT
    weights moving): output lands w-on-partitions, eliminating all 16 T1
    transposes and their evacs; emitted tile-outer so PE progresses as
    each input tile's color completes
  - chroma: horizontal 2x pool on gpsimd into one merged tile; VERTICAL
    2x pool fused into the chroma stage-1 weights; chroma color = 4
    merged STTs using rank-3 APs over the (tile, w) structure; stage-1
    chroma data-stationary as well (no chroma transposes)
  - stage 2 weight-stationary; forward kept fp32 end-to-end (bit-stable
    diff_round decisions); inverse path in f32r (1 cyc/row matmuls,
    1.5 cyc/row transposes)
  - quant via custom fused DVE op: out = r + (q*invT - r)^3 with RNE
    magic; quant/dequant tables stored as [128,8] period-8 patterns read
    through stride-0 outer AP dims (cuts 1.5MB off the const DMA)
  - dequant: one gpsimd tensor_tensor per merged (y / cbcr) tile
  - iB folds the YCbCr->RGB chroma coefficients into 8 pre-scaled weight
    variants; G's two chroma terms accumulate in PSUM, so the old gq
    precombine STT is gone; R,G share one psum tile, evac'd by a single
    wide ACT into one cq3 SBUF tile
  - recombine+clip fused into ONE custom DVE op per channel:
    out = clip01(Y + chroma) with W-upsample via step-0 (dup2) reads
  - fp16 output tile -> halves the store DMA; the last image's final two
    row-blocks store per-channel so the tail store starts after the
    first clip
  - emission is software-pipelined: A(b), iA(b-1)+c3, T2(b-1)+first y
    row-block transpose, M(b), then iB/clips(b-1) with the remaining y
    T2 row-blocks staggered one ahead of their iB consumers -> every
    PE->ACT->PE evac round-trip hides under independent PE work
  - STT on gpsimd builds and simulates, but every color-op rebalance onto
    Pool loses: Pool's in-order queue serializes hpool/dequant ahead of
    the forward-critical color chain (measured +3..+14us) - keep all
    scalar_tensor_tensor on DVE; custom DVE ISA ops and STT have NO
    2x/4x 16-bit perf modes (only TT/TensorCopy do), so the element-wise
    path stays fp32
"""

import math
import os
import re

import numpy as np

import concourse.bacc as bacc
import concourse.bass as bass
import concourse.mybir as mybir
from concourse.mybir import ActivationFunctionType as Act, AluOpType as Op
from concourse.tile import TileContext

# --------------------------------------------------------------------------
# custom DVE op: out = diff_round(Src0 * Src1)
# --------------------------------------------------------------------------
import concourse.dve_ops as dve_ops
from concourse.dve_spec import C0, One, Spec, Src0, Src1, Zero, maxx, minn

MAGIC = float(np.float32(1.5 * 2**23))  # RNE rounding magic for |x| << 2^22


def _diffround_ref(in0, in1, s0, s1, imm2):
    m = (in0.astype(np.float32) * in1.astype(np.float32)).astype(np.float32)
    r = ((m + np.float32(s0)) - np.float32(s0)).astype(np.float32)
    e = (m - r).astype(np.float32)
    return (r + e * e * e).astype(np.float32)


_m = Src0 * Src1
_r = (_m + C0) - C0
_e = _m - _r
_DR_SPEC = Spec(body=_r + _e * _e * _e, reference=_diffround_ref)


def _clip_stt_ref(in0, in1, s0, s1, imm2):
    in1 = np.asarray(in1, np.float32).reshape(in0.shape)
    v = (in0.astype(np.float32) + np.float32(s0) * in1).astype(np.float32)
    return np.minimum(np.maximum(v, np.float32(0.0)), np.float32(1.0))


_CLIP_SPEC = Spec(
    body=minn(maxx(Src0 + C0 * Src1, Zero), One), reference=_clip_stt_ref
)


def _register_custom(name, spec):
    for op in dve_ops.OPS:
        if op.name == name:
            return op
    op = dve_ops.DveOp(name, spec, subdim=False, uops_sha={})
    dve_ops.OPS.append(op)
    dve_ops._SUB_OPCODE_FOR_NAME[name] = (
        dve_ops._CUSTOM_DVE_ROW_BASE + len(dve_ops.OPS) - 1
    )
    dve_ops.CUSTOM_DVE_SPECS[name] = spec
    for ver in ("v3", "v4"):
        try:
            op.compile(ver)
        except ValueError as e:
            m = re.search(r'="([0-9a-f]+)"', str(e))
            if m is None:
                raise
            op.uops_sha[ver] = m.group(1)
            op.compile(ver)
    return op


DIFFROUND = _register_custom("DIFF_ROUND_QANT", _DR_SPEC)
CLIPSTT = _register_custom("STT_CLIP01", _CLIP_SPEC)

# --------------------------------------------------------------------------
# constants
# --------------------------------------------------------------------------
P = 128
DT = mybir.dt.float32
NIMG = 4  # images per core
FACTOR = 0.4
# f32r mode: forward (stage1/stage2) risks diff_round boundary flips; the
# inverse path (iA/iB) is smooth so f32r there is ~1e-4-level noise only.
F32R_FWD = os.environ.get("KERNEL_F32R_FWD", "0") == "1"
F32R_INV = os.environ.get("KERNEL_F32R_INV", "1") == "1"
POOL_ON_GPSIMD = os.environ.get("KERNEL_POOL_GPSIMD", "1") == "1"
COLOR_ON_GPSIMD = os.environ.get("KERNEL_COLOR_GPSIMD", "0") == "1"

# constants packed into three tensors (always-fp32 / forward weights /
# inverse weights) -> one DMA + one sem each; weight groups take the dtype
# of their matmul path so the f32r producer-dtype rule is satisfied.
def _mk_layout(items):
    off_map, off = {}, 0
    for n, w in items:
        off_map[n] = (off, w)
        off += w
    return off_map, off


_CONST_OFF, _CTOT = _mk_layout(
    [
        ("q1y", 8),
        ("p2y", 8),
        ("q1c", 8),
        ("p2c", 8),
    ]
)
_CONSTF_OFF, _CFTOT = _mk_layout(
    [("w_s1yn", 128), ("w_s1cf", 64), ("w_s1cfb", 64), ("w_s1cfr", 64), ("w_s2", 128)]
)
_CONSTI_OFF, _CITOT = _mk_layout(
    [("w_idct", 128)]
    + [(f"w_ib{par}{k}", 128) for par in (0, 1) for k in ("r", "g1", "g2", "b")]
    + [("identi", 128)]
)

# color Horner ratios (float64 -> cast later)
_AY = 0.587 / 0.299
_BY = 0.114 / 0.587
_ACB = -0.331264 / 0.5
_BCB = -0.168736 / 0.5
_RCB = _BCB / _ACB
_ACR = -0.418688 / 0.5
_BCR = -0.081312 / 0.5
_RCR = _BCR / _ACR


def build_const_arrays(y_table, c_table):
    A = np.zeros((8, 8), np.float64)  # A[u,x] = cos((2x+1) u pi/16)
    for u in range(8):
        for x in range(8):
            A[u, x] = math.cos((2 * x + 1) * u * math.pi / 16)
    alpha = np.array([1.0 / math.sqrt(2)] + [1.0] * 7)
    Ah = (0.5 * alpha)[:, None] * A  # Ah[u,x] = 0.5*alpha_u*A[u,x]
    cY = 255.0 * 0.299
    cC = 0.5 * 255.0 / 4.0

    C = {}
    # natural-layout stage-1 Y: partitions = raw rows (16 blocks x 8 x),
    # block-diagonal (Ib,x)->(Ib,u)
    W = np.zeros((128, 128), np.float64)
    for p in range(128):
        Ib, x = p // 8, p % 8
        for u in range(8):
            W[p, 8 * Ib + u] = Ah[u, x] * cY
    C["w_s1yn"] = W
    # chroma stage-1 with vertical 2x pool fused: 128 raw rows ->
    # (8 pooled blocks x 8 u); adjacent row pairs share pooled x'
    W = np.zeros((128, 64), np.float64)
    for p in range(128):
        Ibc, xp = p // 16, (p // 2) % 8
        for u in range(8):
            W[p, 8 * Ibc + u] = Ah[u, xp] * cC
    C["w_s1cf"] = W
    # chroma color STT #2 folded into stage-1: cb = _ACB*t1 + B and
    # cr = _ACR*t2 + R become two-term PSUM accumulations with these
    # pre-scaled weight variants
    C["w_s1cfb"] = _ACB * W
    C["w_s1cfr"] = _ACR * W
    W = np.zeros((128, 128))
    for wl in range(128):
        J, y = wl // 8, wl % 8
        for v in range(8):
            W[wl, 8 * J + v] = Ah[v, y]
    C["w_s2"] = W
    W = np.zeros((128, 128))
    for j in range(16):
        for v in range(8):
            for y in range(8):
                W[8 * j + v, 8 * j + y] = Ah[v, y]
    C["w_idct"] = W
    for par in (0, 1):
        W = np.zeros((128, 128))
        for p in range(128):
            xloc = 64 * par + p // 2
            Ib, x = xloc // 8, xloc % 8
            for u in range(8):
                W[8 * Ib + u, p] = Ah[u, x]
        # color-recombine coefficients folded into the chroma iB weights:
        # r: 1.402*cr; g1/g2: -0.344136*cb - 0.714136*cr (PSUM-accumulated);
        # b: 1.772*cb
        for k, sc in (("r", 1.402), ("g1", -0.344136), ("g2", -0.714136), ("b", 1.772)):
            C[f"w_ib{par}{k}"] = sc * W
    C["identi"] = np.eye(128)

    def pats(T):
        # period-8 tables: row v = p%8, col u; consumers read them with
        # stride-0 outer AP dims to tile across any width
        T = np.asarray(T, np.float64)
        q1 = np.zeros((128, 8))
        p2 = np.zeros((128, 8))
        for p in range(128):
            v = p % 8
            for u in range(8):
                q1[p, u] = 1.0 / (T[u, v] * FACTOR)
                p2[p, u] = T[u, v] * FACTOR / 255.0
        return q1, p2

    C["q1y"], C["p2y"] = pats(y_table)
    C["q1c"], C["p2c"] = pats(c_table)

    def pack(off_map, tot):
        p = np.zeros((128, tot), np.float32)
        for n, (off, w) in off_map.items():
            p[:, off : off + w] = np.asarray(C[n], np.float32)
        return p

    return pack(_CONST_OFF, _CTOT), pack(_CONSTF_OFF, _CFTOT), pack(_CONSTI_OFF, _CITOT)


# --------------------------------------------------------------------------
# program
# --------------------------------------------------------------------------
def build_program():
    FDT = mybir.dt.float32r if F32R_FWD else DT
    IDT = mybir.dt.float32r if F32R_INV else DT
    nc = bacc.Bacc("TRN2", target_bir_lowering=False)
    img = nc.dram_tensor("img", [NIMG, 3, 512, 512], DT, kind="ExternalInput")
    out = nc.dram_tensor(
        "out", [NIMG, 3, 512, 512], mybir.dt.float16, kind="ExternalOutput"
    )
    cdram = nc.dram_tensor("consts", [128, _CTOT], DT, kind="ExternalInput")
    cfdram = nc.dram_tensor("constsf", [128, _CFTOT], FDT, kind="ExternalInput")
    cidram = nc.dram_tensor("constsi", [128, _CITOT], IDT, kind="ExternalInput")

    def mk(ap):
        return ap

    with TileContext(nc) as tc:
        with (
            tc.tile_pool(name="pc", bufs=1) as pc,
            tc.tile_pool(name="ps", bufs=8, space="PSUM") as ps,
            tc.tile_pool(name="pin", bufs=4) as pin,
            tc.tile_pool(name="py", bufs=5) as py,
            tc.tile_pool(name="php", bufs=2) as php,
            tc.tile_pool(name="pcc", bufs=2) as pcc,
            tc.tile_pool(name="pt2s", bufs=7) as pt2s,
            tc.tile_pool(name="pmid", bufs=2) as pmid,
            tc.tile_pool(name="pdeq", bufs=2) as pdeq,
            tc.tile_pool(name="pc3", bufs=6) as pc3,
            tc.tile_pool(name="pc4", bufs=12) as pc4,
            tc.tile_pool(name="pcup", bufs=5) as pcup,
            tc.tile_pool(name="prgb", bufs=6) as prgb,
        ):
            def load_tile(b, t, split=False):
                tl = pin.tile([P, 1536], DT, tag="in", name=f"in{b}_{t}")
                if split:
                    # w-halves: image 0's color chases the DMA so only the
                    # last half's color sits on the stage-1 barrier (finer
                    # splits go below the 625ns HWDGE issue time and become
                    # issue-bound)
                    for h in (0, 1):
                        nc.sync.dma_start(
                            out=tl[:]
                            .rearrange("p (c f) -> p c f", c=3)[
                                :, :, 256 * h : 256 * h + 256
                            ],
                            in_=img[b][
                                :, 128 * t : 128 * t + 128, 256 * h : 256 * h + 256
                            ].rearrange("c h w -> h c w"),
                        )
                else:
                    nc.sync.dma_start(
                        out=tl[:].rearrange("p (c f) -> p c f", c=3),
                        in_=img[b][:, 128 * t : 128 * t + 128, :].rearrange(
                            "c h w -> h c w"
                        ),
                    )
                return tl

            def load_nt(b):
                return {t: load_tile(b, t) for t in range(4)}

            # first image tile ahead of the consts in the DMA queue: color
            # for tile 0 can start while the (later-needed) tables land
            nt0 = {0: load_tile(0, 0, split=True)}

            cwt = pc.tile([128, _CTOT], DT, tag="consts", name="t_consts")
            nc.sync.dma_start(out=cwt[:], in_=cdram[:])
            cwtf = pc.tile([128, _CFTOT], FDT, tag="constsf", name="t_constsf")
            nc.sync.dma_start(out=cwtf[:], in_=cfdram[:])
            cw = {
                n: cwt[:, off : off + w] for n, (off, w) in _CONST_OFF.items()
            }
            cw.update(
                {n: cwtf[:, off : off + w] for n, (off, w) in _CONSTF_OFF.items()}
            )
            # warm DVE/ACT vector clocks past the const DMA so downstream
            # STT/custom-DVE instructions never carry the const-DMA wait
            # (the STT instruction struct encodes at most one sync wait).
            # Emitted lazily AFTER image 0's color ops so the const-DMA wait
            # does not head-of-line block the (const-free) color STTs.
            scr = pc.tile([1, 8], DT, tag="scr", name="scr0")

            def warmup():
                nc.vector.tensor_copy(scr[0:1, 0:1], cwt[0:1, 0:1])
                nc.scalar.activation(scr[0:1, 1:2], cwt[0:1, 0:1], Act.Copy)

            nt0[1] = load_tile(0, 1, split=True)
            cwti = pc.tile([128, _CITOT], IDT, tag="constsi", name="t_constsi")
            nc.sync.dma_start(out=cwti[:], in_=cidram[:])
            cw.update(
                {n: cwti[:, off : off + w] for n, (off, w) in _CONSTI_OFF.items()}
            )
            nt0.update({t: load_tile(0, t, split=True) for t in range(2, 4)})

            eng_pool = nc.gpsimd if POOL_ON_GPSIMD else nc.vector
            eng_col = nc.gpsimd if COLOR_ON_GPSIMD else nc.vector

            import bass_rust as _br

            def reap(ap, dims):
                # keep the partition dim, replace the free dims
                return _br.AP(
                    tensor=ap.tensor,
                    offset=ap.offset,
                    ap=[list(ap.ap[0])] + [list(d) for d in dims],
                )

            def stage_a(b):
                """Load -> color/pool -> stage1 -> T1 (outputs t2s in SBUF)."""
                nt = nt0 if b == 0 else load_nt(b)

                # ---------------- Y color (Horner STT) ----------------
                yt = {}
                for t in range(4):
                    rgb = nt[t]
                    t1 = py.tile([P, 512], DT, tag="yt1", bufs=2, name=f"yt1_{b}{t}")
                    t2 = py.tile([P, 512], FDT, tag="yt2", name=f"yt2_{b}{t}")
                    halves = (
                        (slice(0, 256), slice(256, 512))
                        if b == 0
                        else (slice(0, 512),)
                    )
                    for hs in halves:
                        eng_col.scalar_tensor_tensor(
                            t1[:, hs],
                            rgb[:, 1024 + hs.start : 1024 + hs.stop],
                            _BY,
                            rgb[:, 512 + hs.start : 512 + hs.stop],
                            Op.mult,
                            Op.add,
                        )
                        eng_col.scalar_tensor_tensor(
                            t2[:, hs],
                            t1[:, hs],
                            _AY,
                            rgb[:, hs],
                            Op.mult,
                            Op.add,
                        )
                    yt[t] = t2

                # ---------------- horizontal 2x pooling into ONE merged
                # tile (vertical pool is fused into the chroma stage-1
                # weights) ----------------
                hp = php.tile([P, 3072], DT, tag="hp", bufs=2, name=f"hp{b}")
                for t in range(4):
                    src3 = nt[t][:].rearrange("p (c f) -> p c f", c=3)
                    eng_pool.tensor_tensor(
                        hp[:, 768 * t : 768 * t + 768].rearrange(
                            "p (c f) -> p c f", c=3
                        ),
                        src3[:, :, 0:512:2],
                        src3[:, :, 1:512:2],
                        Op.add,
                    )

                # ---------------- chroma color, merged across tiles via
                # rank-3 APs over the (t, w) structure of hp ----------------
                def hpch(c):  # channel c of all 4 t-blocks: [(t,4),(w,256)]
                    return reap(hp[:, 256 * c : 256 * c + 256], [[768, 4], [1, 256]])

                t1m = pcc.tile([P, 1024], FDT, tag="cct", bufs=4, name=f"cbt{b}")
                eng_col.scalar_tensor_tensor(
                    t1m[:], hpch(0), _RCB, hpch(1), Op.mult, Op.add
                )
                t2m = pcc.tile([P, 1024], FDT, tag="cct", bufs=4, name=f"crt{b}")
                eng_col.scalar_tensor_tensor(
                    t2m[:], hpch(2), _RCR, hpch(1), Op.mult, Op.add
                )
                # (Horner intermediate, its scaled weights, raw hp channel
                # offset): color STT #2 is folded into stage-1 chroma as a
                # second PSUM-accumulated stationary operand
                cbcr = {"cb": (t1m, "w_s1cfb", 512), "cr": (t2m, "w_s1cfr", 0)}
                for ch in ("cb", "cr"):
                    tm, wsc, hoff = cbcr[ch]
                    pt = ps.tile([P, 512], DT, tag="ps", name=f"p_s1{ch}{b}")
                    for jc in range(2):
                        for t in range(4):
                            sl = slice(256 * jc + 64 * t, 256 * jc + 64 * t + 64)
                            nc.tensor.matmul(
                                pt[:, sl],
                                mk(tm[:, 256 * t + 128 * jc : 256 * t + 128 * jc + 128]),
                                mk(cw[wsc][:, 0:64]),
                                start=True,
                                stop=False,
                            )
                            nc.tensor.matmul(
                                pt[:, sl],
                                mk(
                                    hp[
                                        :,
                                        768 * t + hoff + 128 * jc : 768 * t
                                        + hoff
                                        + 128 * jc
                                        + 128,
                                    ]
                                ),
                                mk(cw["w_s1cf"][:, 0:64]),
                                start=False,
                                stop=True,
                            )
                    s = pt2s.tile([P, 512], FDT, tag="t2s", name=f"t2s{ch}{b}")
                    nc.scalar.activation(s[:], pt[:], Act.Copy)
                    t2s[ch] = s

                if b == 0:
                    warmup()

                # ---------------- stage 1 Y (data-stationary: yt chunks as
                # lhsT, w_s1yn moving) -> output lands w-on-partitions, no
                # T1 transpose and no c1 bias (the -128/+128 JPEG shift is
                # an exact integer quant-step offset under RNE, so omitting
                # it forward AND backward cancels bit-exactly) -------------
                t2s = {}
                pts = {
                    j: ps.tile([P, 512], DT, tag="ps", name=f"p_s1y{b}{j}")
                    for j in range(4)
                }
                for t in range(4):
                    for j in range(4):
                        nc.tensor.matmul(
                            pts[j][:, 128 * t : 128 * t + 128],
                            mk(yt[t][:, 128 * j : 128 * j + 128]),
                            mk(cw["w_s1yn"][:]),
                            start=True,
                            stop=True,
                        )
                for j in range(4):
                    s = pt2s.tile([P, 512], FDT, tag="t2s", name=f"t2sy{b}{j}")
                    nc.scalar.activation(s[:], pts[j][:], Act.Copy)
                    t2s["y", j] = s

                # ---------------- stage 1 chroma (data-stationary, output
                # already w-on-partitions -> no chroma transpose); the two
                # jc halves of each channel share one psum tile ----------
                return t2s

            def stage_m1(b, t2s):
                """Stage 2 Y + diff_round Y + dequant Y."""
                ymid_y = pmid.tile([P, 2048], DT, tag="ymidy", bufs=2, name=f"mdy{b}")
                for j in range(4):
                    pt = ps.tile([P, 512], DT, tag="ps", name=f"p_s2y{b}{j}")
                    nc.tensor.matmul(
                        pt[:], mk(cw["w_s2"][:]), mk(t2s["y", j][:]),
                        start=True, stop=True,
                    )
                    nc.vector._custom_dve(
                        DIFFROUND,
                        out=ymid_y[:, 512 * j : 512 * j + 512],
                        in0=pt[:],
                        in1=reap(cw["q1y"][:, 0:8], [[0, 64], [1, 8]]),
                        s0=MAGIC,
                    )
                deq_y = pdeq.tile([P, 2048], IDT, tag="deqy", bufs=2, name=f"dqy{b}")
                nc.gpsimd.tensor_tensor(
                    deq_y[:],
                    ymid_y[:],
                    reap(cw["p2y"][:, 0:8], [[0, 256], [1, 8]]),
                    Op.mult,
                )
                return deq_y

            def stage_m2(b, t2s):
                """Stage 2 chroma + diff_round + dequant."""
                ymid_c = pmid.tile([P, 1024], DT, tag="ymidc", bufs=1, name=f"mdc{b}")
                for ci, ch in enumerate(("cb", "cr")):
                    pt2 = ps.tile([P, 512], DT, tag="ps", name=f"p_s2{ch}{b}")
                    nc.tensor.matmul(
                        pt2[:], mk(cw["w_s2"][:]), mk(t2s[ch][:]),
                        start=True, stop=True,
                    )
                    nc.vector._custom_dve(
                        DIFFROUND,
                        out=ymid_c[:, 512 * ci : 512 * ci + 512],
                        in0=pt2[:],
                        in1=reap(cw["q1c"][:, 0:8], [[0, 64], [1, 8]]),
                        s0=MAGIC,
                    )
                deq_c = pdeq.tile([P, 1024], IDT, tag="deqc", bufs=1, name=f"dqc{b}")
                nc.gpsimd.tensor_tensor(
                    deq_c[:],
                    ymid_c[:],
                    reap(cw["p2c"][:, 0:8], [[0, 128], [1, 8]]),
                    Op.mult,
                )
                return deq_c

            def back_ia(b, deqs):
                """Inverse: iA -> c3 (PE-only consumers of deq; emitted
                before the next image's stage-2 so the c3 ACT evacs hide
                under stage-2's PE work)."""
                deq_y, deq_c = deqs
                # ---------------- iA (inverse W) + c3; chroma first so
                # T2-chroma's c3 inputs evac earliest ----------------
                c3 = {}
                for ci, ch in enumerate(("cb", "cr")):
                    pt = ps.tile([P, 512], DT, tag="ps", name=f"p_ia{ch}{b}")
                    nc.tensor.matmul(
                        pt[:],
                        mk(cw["w_idct"][:]),
                        mk(deq_c[:, 512 * ci : 512 * ci + 512]),
                        start=True,
                        stop=True,
                    )
                    s = pc3.tile([P, 512], IDT, tag="c3", name=f"c3{ch}{b}")
                    nc.scalar.activation(s[:], pt[:], Act.Copy)
                    c3[ch] = s
                for j in range(4):
                    pt = ps.tile([P, 512], DT, tag="ps", name=f"p_iay{b}{j}")
                    nc.tensor.matmul(
                        pt[:],
                        mk(cw["w_idct"][:]),
                        mk(deq_y[:, 512 * j : 512 * j + 512]),
                        start=True,
                        stop=True,
                    )
                    s = pc3.tile([P, 512], IDT, tag="c3", name=f"c3y{b}{j}")
                    nc.scalar.activation(s[:], pt[:], Act.Copy)
                    c3["y", j] = s
                return c3

            def back_t2(b, c3):
                """T2 transpose + c4 (chroma only; y row-blocks are
                staggered inside back_b)."""
                c4 = {}
                for ch in ("cb", "cr"):
                    pt = ps.tile([P, 512], IDT, tag="ps", name=f"p_t2{ch}{b}")
                    for mp in range(2):
                        for jc in range(2):
                            nc.tensor.transpose(
                                pt[:, 256 * mp + 128 * jc : 256 * mp + 128 * jc + 128],
                                c3[ch][:, 256 * jc + 128 * mp : 256 * jc + 128 * mp + 128],
                                cw["identi"][:],
                            )
                    s = pc4.tile([P, 512], IDT, tag="c4", name=f"c4{ch}{b}")
                    nc.scalar.activation(s[:], pt[:], Act.Copy)
                    c4[ch] = s
                c4["c3y"] = [c3["y", j] for j in range(4)]
                return c4

            def t2y(b, c4, mo):
                """One y row-block of T2 + its c4 evac."""
                c3y = c4["c3y"]
                pt = ps.tile([P, 512], IDT, tag="ps", name=f"p_t2y{b}{mo}")
                for j in range(4):
                    nc.tensor.transpose(
                        pt[:, 128 * j : 128 * j + 128],
                        c3y[j][:, 128 * mo : 128 * mo + 128],
                        cw["identi"][:],
                    )
                c4y = pc4.tile([P, 512], IDT, tag="c4", name=f"c4y{b}{mo}")
                nc.scalar.activation(c4y[:], pt[:], Act.Copy)
                return c4y

            def back_b(items, split_last=False, mo_range=range(4)):
                """Inverse second half for one or more images, row-block
                interleaved: T2-y(m) -> iB (color coeffs folded into chroma
                weights; G precombined by PSUM accumulation) -> chroma evac
                -> fused upsample/recombine/clip -> store. T2-y is emitted
                one row-block ahead of its iB consumer so the PE never
                stalls on the c4 evac round-trip."""
                c4ys = {b: c4.pop("y0") if "y0" in c4 else t2y(b, c4, 0) for b, c4 in items}
                for mo in mo_range:
                    for b, c4 in items:
                        c4y = c4ys[b]
                        if mo < 3:
                            c4ys[b] = t2y(b, c4, mo + 1)
                            c4["y0"] = c4ys[b]
                        ypt = ps.tile([P, 512], DT, tag="ps", name=f"p_iby{b}{mo}")
                        nc.tensor.matmul(
                            ypt[:],
                            mk(cw["w_idct"][:]),
                            mk(c4y[:]),
                            start=True,
                            stop=True,
                        )
                        par = mo % 2
                        mps = slice(256 * (mo // 2), 256 * (mo // 2) + 256)
                        # R,G chroma in one psum tile; B in a second
                        crg = ps.tile([P, 512], DT, tag="ps", name=f"p_ibrg{b}{mo}")
                        nc.tensor.matmul(
                            crg[:, 0:256],
                            mk(cw[f"w_ib{par}r"][:]),
                            mk(c4["cr"][:, mps]),
                            start=True,
                            stop=True,
                        )
                        nc.tensor.matmul(
                            crg[:, 256:512],
                            mk(cw[f"w_ib{par}g1"][:]),
                            mk(c4["cb"][:, mps]),
                            start=True,
                            stop=False,
                        )
                        nc.tensor.matmul(
                            crg[:, 256:512],
                            mk(cw[f"w_ib{par}g2"][:]),
                            mk(c4["cr"][:, mps]),
                            start=False,
                            stop=True,
                        )
                        cbp = ps.tile([P, 256], DT, tag="ps", name=f"p_ibb{b}{mo}")
                        nc.tensor.matmul(
                            cbp[:],
                            mk(cw[f"w_ib{par}b"][:]),
                            mk(c4["cb"][:, mps]),
                            start=True,
                            stop=True,
                        )
                        # evac all chroma into one SBUF tile (R,G,B blocks)
                        cq3 = pcup.tile([P, 768], DT, tag="cup", name=f"cu{b}{mo}")
                        nc.scalar.activation(cq3[:, 0:512], crg[:], Act.Copy)
                        nc.scalar.activation(cq3[:, 512:768], cbp[:], Act.Copy)

                        rows = slice(128 * mo, 128 * mo + 128)
                        # absorber: pull the PE-sem wait onto a 1x1 copy so
                        # the custom op carries at most one sync wait
                        ab = prgb.tile([1, 1], DT, tag="ab", name=f"ab{b}{mo}")
                        nc.vector.tensor_copy(ab[0:1, 0:1], ypt[0:1, 0:1])
                        # fused upsample/recombine/clip per channel:
                        # out[2j+k] = clip01(Y[2j+k] + cq3[256c + j])
                        rgb16 = prgb.tile(
                            [P, 1536],
                            mybir.dt.float16,
                            tag="rgb16",
                            bufs=4,
                            name=f"rgb16_{b}{mo}",
                        )
                        for c in range(3):
                            nc.vector._custom_dve(
                                CLIPSTT,
                                out=rgb16[:, 512 * c : 512 * c + 512],
                                in0=ypt[:],
                                in1=reap(
                                    cq3[:, 256 * c : 256 * c + 256],
                                    [[1, 256], [0, 2]],
                                ),
                                s0=1.0,
                            )
                        if split_last and mo >= 2:
                            # per-channel stores: the tail store begins after
                            # the first clip instead of after all three
                            for c in range(3):
                                nc.sync.dma_start(
                                    out=out[b][c, rows, :],
                                    in_=rgb16[:, 512 * c : 512 * c + 512],
                                )
                        else:
                            nc.sync.dma_start(
                                out=out[b][:, rows, :].rearrange("c h w -> h c w"),
                                in_=rgb16[:].rearrange("p (c f) -> p c f", c=3),
                            )

            # software-pipelined emission: A(b)+M(b) | bA(b-1)+bB(b-1) keeps
            # each engine's in-order queue interleaving two images; the final
            # two images' iB halves are row-block interleaved so the drain
            # keeps PE/ACT/DVE all fed until the last store
            pm = {}
            for b in range(NIMG):
                a = stage_a(b)
                dy = stage_m1(b, a)
                if b >= 1:
                    c3 = back_ia(b - 1, pm.pop(b - 1))
                dc = stage_m2(b, a)
                pm[b] = (dy, dc)
                if b >= 1:
                    st = back_t2(b - 1, c3)
                    st["y0"] = t2y(b - 1, st, 0)
                    back_b([(b - 1, st)])
            stl = back_t2(NIMG - 1, back_ia(NIMG - 1, pm.pop(NIMG - 1)))
            back_b([(NIMG - 1, stl)], split_last=True)

    nc.compile()
    return nc


# --------------------------------------------------------------------------
# entry point
# --------------------------------------------------------------------------
_last_results = None


_nc_cache = None


def kernel(image, y_table, c_table):
    global _last_results, _nc_cache
    from concourse import bass_utils

    image = np.ascontiguousarray(np.asarray(image), np.float32)
    packed, packedf, packedi = build_const_arrays(
        np.asarray(y_table), np.asarray(c_table)
    )

    # the program is table-independent (tables arrive as data through the
    # consts DRAM tensors), so repeat kernel() calls reuse the built module
    if _nc_cache is None:
        _nc_cache = build_program()
    nc = _nc_cache
    n_cores = 8
    per = image.shape[0] // n_cores
    in_maps = [
        {
            "img": np.ascontiguousarray(image[i * per : (i + 1) * per]),
            "consts": packed,
            "constsf": packedf,
            "constsi": packedi,
        }
        for i in range(n_cores)
    ]

    res = None
    last_exc = None
    for attempt in range(3):
        try:
            res = bass_utils.run_bass_kernel_spmd(
                nc,
                in_maps,
                core_ids=list(range(n_cores)),
                trace=os.environ.get("KERNEL_TRACE", "0") == "1",
            )
            break
        except Exception as e:  # transient NRT/device hiccups: retry
            last_exc = e
    if res is None:
        raise last_exc
    _last_results = res
    outs = [np.asarray(r["out"], np.float32) for r in res.results]
    return np.concatenate(outs, axis=0)


if __name__ == "__main__":
    rng = np.random.default_rng(0)
    img = rng.random((32, 3, 512, 512), np.float32)
    yt = np.ones((8, 8), np.float32)
    ct = np.ones((8, 8), np.float32)
    out = kernel(img, yt, ct)
    print("out", out.shape, out.dtype, float(out.min()), float(out.max()))



# revision 88
# speedup vs baseline: 1.3681x; 1.3681x over previous
"""DiffJPEG Trainium2 Bass kernel.

Strategy (pure data-parallel over batch, 4 images per core on 8 cores):
  - load RGB in natural row layout [128 rows, 3x512] (x = row%8 fully on
    partitions); ALL of image 0's tile DMAs are split into two w-halves
    with per-half Y-color so only the last half's color sits on the
    stage-1 barrier (finer splits drop below the 625ns HWDGE issue time
    and become issue-bound)
  - RGB->Y via 2 fused scalar_tensor_tensor (Horner) on DVE
  - the JPEG -128/+128 shift is OMITTED on both sides: an integer shift
    of the DC coefficient by exactly -160 quant steps commutes with RNE
    rounding (round(x-160) == round(x)-160 and the diff_round cubic is
    shift-invariant), so forward-shift and inverse-add-back cancel
    bit-exactly and no ACT bias vectors are needed anywhere
  - stage-1 Y is DATA-STATIONARY (yt chunks as lhsT, block-diag DCT
    weights moving): output lands w-on-partitions, eliminating all 16 T1
    transposes and their evacs; emitted tile-outer so PE progresses as
    each input tile's color completes
  - chroma: horizontal 2x pool on gpsimd into one merged tile; VERTICAL
    2x pool fused into the chroma stage-1 weights; chroma color = 4
    merged STTs using rank-3 APs over the (tile, w) structure; stage-1
    chroma data-stationary as well (no chroma transposes)
  - stage 2 weight-stationary; forward kept fp32 end-to-end (bit-stable
    diff_round decisions); inverse path in f32r (1 cyc/row matmuls,
    1.5 cyc/row transposes)
  - quant via custom fused DVE op: out = r + (q*invT - r)^3 with RNE
    magic; quant/dequant tables stored as [128,8] period-8 patterns read
    through stride-0 outer AP dims (cuts 1.5MB off the const DMA)
  - dequant: one gpsimd tensor_tensor per merged (y / cbcr) tile
  - iB folds the YCbCr->RGB chroma coefficients into 8 pre-scaled weight
    variants; G's two chroma terms accumulate in PSUM, so the old gq
    precombine STT is gone; R,G share one psum tile, evac'd by a single
    wide ACT into one cq3 SBUF tile
  - recombine+clip fused into ONE custom DVE op per channel:
    out = clip01(Y + chroma) with W-upsample via step-0 (dup2) reads
  - fp16 output tile -> halves the store DMA; the last image's final two
    row-blocks store per-channel so the tail store starts after the
    first clip
  - emission is software-pipelined: A(b), iA(b-1)+c3, T2(b-1)+first y
    row-block transpose, M(b), then iB/clips(b-1) with the remaining y
    T2 row-blocks staggered one ahead of their iB consumers -> every
    PE->ACT->PE evac round-trip hides under independent PE work
  - STT on gpsimd builds and simulates, but every color-op rebalance onto
    Pool loses: Pool's in-order queue serializes hpool/dequant ahead of
    the forward-critical color chain (measured +3..+14us) - keep all
    scalar_tensor_tensor on DVE; custom DVE ISA ops and STT have NO
    2x/4x 16-bit perf modes (only TT/TensorCopy do), so the element-wise
    path stays fp32
"""

import math
import os
import re

import numpy as np

import concourse.bacc as bacc
import concourse.bass as bass
import concourse.mybir as mybir
from concourse.mybir import ActivationFunctionType as Act, AluOpType as Op
from concourse.tile import TileContext

# --------------------------------------------------------------------------
# custom DVE op: out = diff_round(Src0 * Src1)
# --------------------------------------------------------------------------
import concourse.dve_ops as dve_ops
from concourse.dve_spec import C0, One, Spec, Src0, Src1, Zero, maxx, minn

MAGIC = float(np.float32(1.5 * 2**23))  # RNE rounding magic for |x| << 2^22


def _diffround_ref(in0, in1, s0, s1, imm2):
    m = (in0.astype(np.float32) * in1.astype(np.float32)).astype(np.float32)
    r = ((m + np.float32(s0)) - np.float32(s0)).astype(np.float32)
    e = (m - r).astype(np.float32)
    return (r + e * e * e).astype(np.float32)


_m = Src0 * Src1
_r = (_m + C0) - C0
_e = _m - _r
_DR_SPEC = Spec(body=_r + _e * _e * _e, reference=_diffround_ref)


def _clip_stt_ref(in0, in1, s0, s1, imm2):
    in1 = np.asarray(in1, np.float32).reshape(in0.shape)
    v = (in0.astype(np.float32) + np.float32(s0) * in1).astype(np.float32)
    return np.minimum(np.maximum(v, np.float32(0.0)), np.float32(1.0))


_CLIP_SPEC = Spec(
    body=minn(maxx(Src0 + C0 * Src1, Zero), One), reference=_clip_stt_ref
)


def _register_custom(name, spec):
    for op in dve_ops.OPS:
        if op.name == name:
            return op
    op = dve_ops.DveOp(name, spec, subdim=False, uops_sha={})
    dve_ops.OPS.append(op)
    dve_ops._SUB_OPCODE_FOR_NAME[name] = (
        dve_ops._CUSTOM_DVE_ROW_BASE + len(dve_ops.OPS) - 1
    )
    dve_ops.CUSTOM_DVE_SPECS[name] = spec
    for ver in ("v3", "v4"):
        try:
            op.compile(ver)
        except ValueError as e:
            m = re.search(r'="([0-9a-f]+)"', str(e))
            if m is None:
                raise
            op.uops_sha[ver] = m.group(1)
            op.compile(ver)
    return op


DIFFROUND = _register_custom("DIFF_ROUND_QANT", _DR_SPEC)
CLIPSTT = _register_custom("STT_CLIP01", _CLIP_SPEC)

# --------------------------------------------------------------------------
# constants
# --------------------------------------------------------------------------
P = 128
DT = mybir.dt.float32
NIMG = 4  # images per core
FACTOR = 0.4
# f32r mode: forward (stage1/stage2) risks diff_round boundary flips; the
# inverse path (iA/iB) is smooth so f32r there is ~1e-4-level noise only.
F32R_FWD = os.environ.get("KERNEL_F32R_FWD", "0") == "1"
F32R_INV = os.environ.get("KERNEL_F32R_INV", "1") == "1"
POOL_ON_GPSIMD = os.environ.get("KERNEL_POOL_GPSIMD", "1") == "1"
COLOR_ON_GPSIMD = os.environ.get("KERNEL_COLOR_GPSIMD", "0") == "1"

# constants packed into three tensors (always-fp32 / forward weights /
# inverse weights) -> one DMA + one sem each; weight groups take the dtype
# of their matmul path so the f32r producer-dtype rule is satisfied.
def _mk_layout(items):
    off_map, off = {}, 0
    for n, w in items:
        off_map[n] = (off, w)
        off += w
    return off_map, off


_CONST_OFF, _CTOT = _mk_layout(
    [
        ("q1y", 8),
        ("p2y", 8),
        ("q1c", 8),
        ("p2c", 8),
    ]
)
_CONSTF_OFF, _CFTOT = _mk_layout(
    [("w_s1yn", 128), ("w_s1cf", 64), ("w_s1cfb", 64), ("w_s1cfr", 64), ("w_s2", 128)]
)
_CONSTI_OFF, _CITOT = _mk_layout(
    [("w_idct", 128)]
    + [(f"w_ib{par}{k}", 128) for par in (0, 1) for k in ("r", "g1", "g2", "b")]
    + [("identi", 128)]
)

# color Horner ratios (float64 -> cast later)
_AY = 0.587 / 0.299
_BY = 0.114 / 0.587
_ACB = -0.331264 / 0.5
_BCB = -0.168736 / 0.5
_RCB = _BCB / _ACB
_ACR = -0.418688 / 0.5
_BCR = -0.081312 / 0.5
_RCR = _BCR / _ACR


def build_const_arrays(y_table, c_table):
    A = np.zeros((8, 8), np.float64)  # A[u,x] = cos((2x+1) u pi/16)
    for u in range(8):
        for x in range(8):
            A[u, x] = math.cos((2 * x + 1) * u * math.pi / 16)
    alpha = np.array([1.0 / math.sqrt(2)] + [1.0] * 7)
    Ah = (0.5 * alpha)[:, None] * A  # Ah[u,x] = 0.5*alpha_u*A[u,x]
    cY = 255.0 * 0.299
    cC = 0.5 * 255.0 / 4.0

    C = {}
    # natural-layout stage-1 Y: partitions = raw rows (16 blocks x 8 x),
    # block-diagonal (Ib,x)->(Ib,u)
    W = np.zeros((128, 128), np.float64)
    for p in range(128):
        Ib, x = p // 8, p % 8
        for u in range(8):
            W[p, 8 * Ib + u] = Ah[u, x] * cY
    C["w_s1yn"] = W
    # chroma stage-1 with vertical 2x pool fused: 128 raw rows ->
    # (8 pooled blocks x 8 u); adjacent row pairs share pooled x'
    W = np.zeros((128, 64), np.float64)
    for p in range(128):
        Ibc, xp = p // 16, (p // 2) % 8
        for u in range(8):
            W[p, 8 * Ibc + u] = Ah[u, xp] * cC
    C["w_s1cf"] = W
    # chroma color STT #2 folded into stage-1: cb = _ACB*t1 + B and
    # cr = _ACR*t2 + R become two-term PSUM accumulations with these
    # pre-scaled weight variants
    C["w_s1cfb"] = _ACB * W
    C["w_s1cfr"] = _ACR * W
    W = np.zeros((128, 128))
    for wl in range(128):
        J, y = wl // 8, wl % 8
        for v in range(8):
            W[wl, 8 * J + v] = Ah[v, y]
    C["w_s2"] = W
    W = np.zeros((128, 128))
    for j in range(16):
        for v in range(8):
            for y in range(8):
                W[8 * j + v, 8 * j + y] = Ah[v, y]
    C["w_idct"] = W
    for par in (0, 1):
        W = np.zeros((128, 128))
        for p in range(128):
            xloc = 64 * par + p // 2
            Ib, x = xloc // 8, xloc % 8
            for u in range(8):
                W[8 * Ib + u, p] = Ah[u, x]
        # color-recombine coefficients folded into the chroma iB weights:
        # r: 1.402*cr; g1/g2: -0.344136*cb - 0.714136*cr (PSUM-accumulated);
        # b: 1.772*cb
        for k, sc in (("r", 1.402), ("g1", -0.344136), ("g2", -0.714136), ("b", 1.772)):
            C[f"w_ib{par}{k}"] = sc * W
    C["identi"] = np.eye(128)

    def pats(T):
        # period-8 tables: row v = p%8, col u; consumers read them with
        # stride-0 outer AP dims to tile across any width
        T = np.asarray(T, np.float64)
        q1 = np.zeros((128, 8))
        p2 = np.zeros((128, 8))
        for p in range(128):
            v = p % 8
            for u in range(8):
                q1[p, u] = 1.0 / (T[u, v] * FACTOR)
                p2[p, u] = T[u, v] * FACTOR / 255.0
        return q1, p2

    C["q1y"], C["p2y"] = pats(y_table)
    C["q1c"], C["p2c"] = pats(c_table)

    def pack(off_map, tot):
        p = np.zeros((128, tot), np.float32)
        for n, (off, w) in off_map.items():
            p[:, off : off + w] = np.asarray(C[n], np.float32)
        return p

    return pack(_CONST_OFF, _CTOT), pack(_CONSTF_OFF, _CFTOT), pack(_CONSTI_OFF, _CITOT)


# --------------------------------------------------------------------------
# program
# --------------------------------------------------------------------------
def build_program():
    FDT = mybir.dt.float32r if F32R_FWD else DT
    IDT = mybir.dt.float32r if F32R_INV else DT
    nc = bacc.Bacc("TRN2", target_bir_lowering=False)
    img = nc.dram_tensor("img", [NIMG, 3, 512, 512], DT, kind="ExternalInput")
    out = nc.dram_tensor(
        "out", [NIMG, 3, 512, 512], mybir.dt.float16, kind="ExternalOutput"
    )
    cdram = nc.dram_tensor("consts", [128, _CTOT], DT, kind="ExternalInput")
    cfdram = nc.dram_tensor("constsf", [128, _CFTOT], FDT, kind="ExternalInput")
    cidram = nc.dram_tensor("constsi", [128, _CITOT], IDT, kind="ExternalInput")

    def mk(ap):
        return ap

    with TileContext(nc) as tc:
        with (
            tc.tile_pool(name="pc", bufs=1) as pc,
            tc.tile_pool(name="ps", bufs=8, space="PSUM") as ps,
            tc.tile_pool(name="pin", bufs=4) as pin,
            tc.tile_pool(name="py", bufs=5) as py,
            tc.tile_pool(name="php", bufs=2) as php,
            tc.tile_pool(name="pcc", bufs=2) as pcc,
            tc.tile_pool(name="pt2s", bufs=7) as pt2s,
            tc.tile_pool(name="pmid", bufs=2) as pmid,
            tc.tile_pool(name="pdeq", bufs=2) as pdeq,
            tc.tile_pool(name="pc3", bufs=6) as pc3,
            tc.tile_pool(name="pc4", bufs=12) as pc4,
            tc.tile_pool(name="pcup", bufs=5) as pcup,
            tc.tile_pool(name="prgb", bufs=6) as prgb,
        ):
            def load_tile(b, t, split=False):
                tl = pin.tile([P, 1536], DT, tag="in", name=f"in{b}_{t}")
                if split:
                    # w-halves: image 0's color chases the DMA so only the
                    # last half's color sits on the stage-1 barrier (finer
                    # splits go below the 625ns HWDGE issue time and become
                    # issue-bound)
                    for h in (0, 1):
                        nc.sync.dma_start(
                            out=tl[:]
                            .rearrange("p (c f) -> p c f", c=3)[
                                :, :, 256 * h : 256 * h + 256
                            ],
                            in_=img[b][
                                :, 128 * t : 128 * t + 128, 256 * h : 256 * h + 256
                            ].rearrange("c h w -> h c w"),
                        )
                else:
                    nc.sync.dma_start(
                        out=tl[:].rearrange("p (c f) -> p c f", c=3),
                        in_=img[b][:, 128 * t : 128 * t + 128, :].rearrange(
                            "c h w -> h c w"
                        ),
                    )
                return tl

            def load_nt(b):
                return {t: load_tile(b, t) for t in range(4)}

            # first image tile ahead of the consts in the DMA queue: color
            # for tile 0 can start while the (later-needed) tables land
            nt0 = {0: load_tile(0, 0, split=True)}

            cwt = pc.tile([128, _CTOT], DT, tag="consts", name="t_consts")
            nc.sync.dma_start(out=cwt[:], in_=cdram[:])
            cwtf = pc.tile([128, _CFTOT], FDT, tag="constsf", name="t_constsf")
            nc.sync.dma_start(out=cwtf[:], in_=cfdram[:])
            cw = {
                n: cwt[:, off : off + w] for n, (off, w) in _CONST_OFF.items()
            }
            cw.update(
                {n: cwtf[:, off : off + w] for n, (off, w) in _CONSTF_OFF.items()}
            )
            # warm DVE/ACT vector clocks past the const DMA so downstream
            # STT/custom-DVE instructions never carry the const-DMA wait
            # (the STT instruction struct encodes at most one sync wait).
            # Emitted lazily AFTER image 0's color ops so the const-DMA wait
            # does not head-of-line block the (const-free) color STTs.
            scr = pc.tile([1, 8], DT, tag="scr", name="scr0")

            def warmup():
                nc.vector.tensor_copy(scr[0:1, 0:1], cwt[0:1, 0:1])
                nc.scalar.activation(scr[0:1, 1:2], cwt[0:1, 0:1], Act.Copy)

            nt0[1] = load_tile(0, 1, split=True)
            cwti = pc.tile([128, _CITOT], IDT, tag="constsi", name="t_constsi")
            nc.sync.dma_start(out=cwti[:], in_=cidram[:])
            cw.update(
                {n: cwti[:, off : off + w] for n, (off, w) in _CONSTI_OFF.items()}
            )
            nt0.update({t: load_tile(0, t, split=True) for t in range(2, 4)})

            eng_pool = nc.gpsimd if POOL_ON_GPSIMD else nc.vector
            eng_col = nc.gpsimd if COLOR_ON_GPSIMD else nc.vector

            import bass_rust as _br

            def reap(ap, dims):
                # keep the partition dim, replace the free dims
                return _br.AP(
                    tensor=ap.tensor,
                    offset=ap.offset,
                    ap=[list(ap.ap[0])] + [list(d) for d in dims],
                )

            def stage_a(b):
                """Load -> color/pool -> stage1 -> T1 (outputs t2s in SBUF)."""
                nt = nt0 if b == 0 else load_nt(b)

                # ---------------- Y color (Horner STT) ----------------
                yt = {}
                for t in range(4):
                    rgb = nt[t]
                    t1 = py.tile([P, 512], DT, tag="yt1", bufs=2, name=f"yt1_{b}{t}")
                    t2 = py.tile([P, 512], FDT, tag="yt2", name=f"yt2_{b}{t}")
                    halves = (
                        (slice(0, 256), slice(256, 512))
                        if b == 0
                        else (slice(0, 512),)
                    )
                    for hs in halves:
                        eng_col.scalar_tensor_tensor(
                            t1[:, hs],
                            rgb[:, 1024 + hs.start : 1024 + hs.stop],
                            _BY,
                            rgb[:, 512 + hs.start : 512 + hs.stop],
                            Op.mult,
                            Op.add,
                        )
                        eng_col.scalar_tensor_tensor(
                            t2[:, hs],
                            t1[:, hs],
                            _AY,
                            rgb[:, hs],
                            Op.mult,
                            Op.add,
                        )
                    yt[t] = t2

                # ---------------- horizontal 2x pooling into ONE merged
                # tile (vertical pool is fused into the chroma stage-1
                # weights) ----------------
                hp = php.tile([P, 3072], DT, tag="hp", bufs=2, name=f"hp{b}")
                for t in range(4):
                    src3 = nt[t][:].rearrange("p (c f) -> p c f", c=3)
                    eng_pool.tensor_tensor(
                        hp[:, 768 * t : 768 * t + 768].rearrange(
                            "p (c f) -> p c f", c=3
                        ),
                        src3[:, :, 0:512:2],
                        src3[:, :, 1:512:2],
                        Op.add,
                    )

                # ---------------- chroma color, merged across tiles via
                # rank-3 APs over the (t, w) structure of hp ----------------
                def hpch(c):  # channel c of all 4 t-blocks: [(t,4),(w,256)]
                    return reap(hp[:, 256 * c : 256 * c + 256], [[768, 4], [1, 256]])

                t1m = pcc.tile([P, 1024], FDT, tag="cct", bufs=4, name=f"cbt{b}")
                eng_col.scalar_tensor_tensor(
                    t1m[:], hpch(0), _RCB, hpch(1), Op.mult, Op.add
                )
                t2m = pcc.tile([P, 1024], FDT, tag="cct", bufs=4, name=f"crt{b}")
                eng_col.scalar_tensor_tensor(
                    t2m[:], hpch(2), _RCR, hpch(1), Op.mult, Op.add
                )
                # (Horner intermediate, its scaled weights, raw hp channel
                # offset): color STT #2 is folded into stage-1 chroma as a
                # second PSUM-accumulated stationary operand
                cbcr = {"cb": (t1m, "w_s1cfb", 512), "cr": (t2m, "w_s1cfr", 0)}
                for ch in ("cb", "cr"):
                    tm, wsc, hoff = cbcr[ch]
                    pt = ps.tile([P, 512], DT, tag="ps", name=f"p_s1{ch}{b}")
                    for jc in range(2):
                        for t in range(4):
                            sl = slice(256 * jc + 64 * t, 256 * jc + 64 * t + 64)
                            nc.tensor.matmul(
                                pt[:, sl],
                                mk(tm[:, 256 * t + 128 * jc : 256 * t + 128 * jc + 128]),
                                mk(cw[wsc][:, 0:64]),
                                start=True,
                                stop=False,
                            )
                            nc.tensor.matmul(
                                pt[:, sl],
                                mk(
                                    hp[
                                        :,
                                        768 * t + hoff + 128 * jc : 768 * t
                                        + hoff
                                        + 128 * jc
                                        + 128,
                                    ]
                                ),
                                mk(cw["w_s1cf"][:, 0:64]),
                                start=False,
                                stop=True,
                            )
                    s = pt2s.tile([P, 512], FDT, tag="t2s", name=f"t2s{ch}{b}")
                    nc.scalar.activation(s[:], pt[:], Act.Copy)
                    t2s[ch] = s

                if b == 0:
                    warmup()

                # ---------------- stage 1 Y (data-stationary: yt chunks as
                # lhsT, w_s1yn moving) -> output lands w-on-partitions, no
                # T1 transpose and no c1 bias (the -128/+128 JPEG shift is
                # an exact integer quant-step offset under RNE, so omitting
                # it forward AND backward cancels bit-exactly) -------------
                t2s = {}
                pts = {
                    j: ps.tile([P, 512], DT, tag="ps", name=f"p_s1y{b}{j}")
                    for j in range(4)
                }
                for t in range(4):
                    for j in range(4):
                        nc.tensor.matmul(
                            pts[j][:, 128 * t : 128 * t + 128],
                            mk(yt[t][:, 128 * j : 128 * j + 128]),
                            mk(cw["w_s1yn"][:]),
                            start=True,
                            stop=True,
                        )
                for j in range(4):
                    s = pt2s.tile([P, 512], FDT, tag="t2s", name=f"t2sy{b}{j}")
                    nc.scalar.activation(s[:], pts[j][:], Act.Copy)
                    t2s["y", j] = s

                # ---------------- stage 1 chroma (data-stationary, output
                # already w-on-partitions -> no chroma transpose); the two
                # jc halves of each channel share one psum tile ----------
                return t2s

            def stage_m1(b, t2s):
                """Stage 2 Y + diff_round Y + dequant Y."""
                ymid_y = pmid.tile([P, 2048], DT, tag="ymidy", bufs=2, name=f"mdy{b}")
                for j in range(4):
                    pt = ps.tile([P, 512], DT, tag="ps", name=f"p_s2y{b}{j}")
                    nc.tensor.matmul(
                        pt[:], mk(cw["w_s2"][:]), mk(t2s["y", j][:]),
                        start=True, stop=True,
                    )
                    nc.vector._custom_dve(
                        DIFFROUND,
                        out=ymid_y[:, 512 * j : 512 * j + 512],
                        in0=pt[:],
                        in1=reap(cw["q1y"][:, 0:8], [[0, 64], [1, 8]]),
                        s0=MAGIC,
                    )
                deq_y = pdeq.tile([P, 2048], IDT, tag="deqy", bufs=2, name=f"dqy{b}")
                nc.gpsimd.tensor_tensor(
                    deq_y[:],
                    ymid_y[:],
                    reap(cw["p2y"][:, 0:8], [[0, 256], [1, 8]]),
                    Op.mult,
                )
                return deq_y

            def stage_m2(b, t2s):
                """Stage 2 chroma + diff_round + dequant."""
                ymid_c = pmid.tile([P, 1024], DT, tag="ymidc", bufs=1, name=f"mdc{b}")
                for ci, ch in enumerate(("cb", "cr")):
                    pt2 = ps.tile([P, 512], DT, tag="ps", name=f"p_s2{ch}{b}")
                    nc.tensor.matmul(
                        pt2[:], mk(cw["w_s2"][:]), mk(t2s[ch][:]),
                        start=True, stop=True,
                    )
                    nc.vector._custom_dve(
                        DIFFROUND,
                        out=ymid_c[:, 512 * ci : 512 * ci + 512],
                        in0=pt2[:],
                        in1=reap(cw["q1c"][:, 0:8], [[0, 64], [1, 8]]),
                        s0=MAGIC,
                    )
                deq_c = pdeq.tile([P, 1024], IDT, tag="deqc", bufs=1, name=f"dqc{b}")
                nc.gpsimd.tensor_tensor(
                    deq_c[:],
                    ymid_c[:],
                    reap(cw["p2c"][:, 0:8], [[0, 128], [1, 8]]),
                    Op.mult,
                )
                return deq_c

            def back_ia(b, deqs):
                """Inverse: iA -> c3 (PE-only consumers of deq; emitted
                before the next image's stage-2 so the c3 ACT evacs hide
                under stage-2's PE work)."""
                deq_y, deq_c = deqs
                # ---------------- iA (inverse W) + c3; chroma first so
                # T2-chroma's c3 inputs evac earliest ----------------
                c3 = {}
                for ci, ch in enumerate(("cb", "cr")):
                    pt = ps.tile([P, 512], DT, tag="ps", name=f"p_ia{ch}{b}")
                    nc.tensor.matmul(
                        pt[:],
                        mk(cw["w_idct"][:]),
                        mk(deq_c[:, 512 * ci : 512 * ci + 512]),
                        start=True,
                        stop=True,
                    )
                    s = pc3.tile([P, 512], IDT, tag="c3", name=f"c3{ch}{b}")
                    nc.scalar.activation(s[:], pt[:], Act.Copy)
                    c3[ch] = s
                for j in range(4):
                    pt = ps.tile([P, 512], DT, tag="ps", name=f"p_iay{b}{j}")
                    nc.tensor.matmul(
                        pt[:],
                        mk(cw["w_idct"][:]),
                        mk(deq_y[:, 512 * j : 512 * j + 512]),
                        start=True,
                        stop=True,
                    )
                    s = pc3.tile([P, 512], IDT, tag="c3", name=f"c3y{b}{j}")
                    nc.scalar.activation(s[:], pt[:], Act.Copy)
                    c3["y", j] = s
                return c3

            def back_t2(b, c3):
                """T2 transpose + c4 (chroma only; y row-blocks are
                staggered inside back_b)."""
                c4 = {}
                for ch in ("cb", "cr"):
                    pt = ps.tile([P, 512], IDT, tag="ps", name=f"p_t2{ch}{b}")
                    for mp in range(2):
                        for jc in range(2):
                            nc.tensor.transpose(
                                pt[:, 256 * mp + 128 * jc : 256 * mp + 128 * jc + 128],
                                c3[ch][:, 256 * jc + 128 * mp : 256 * jc + 128 * mp + 128],
                                cw["identi"][:],
                            )
                    s = pc4.tile([P, 512], IDT, tag="c4", name=f"c4{ch}{b}")
                    nc.scalar.activation(s[:], pt[:], Act.Copy)
                    c4[ch] = s
                c4["c3y"] = [c3["y", j] for j in range(4)]
                return c4

            def t2y(b, c4, mo):
                """One y row-block of T2 + its c4 evac."""
                c3y = c4["c3y"]
                pt = ps.tile([P, 512], IDT, tag="ps", name=f"p_t2y{b}{mo}")
                for j in range(4):
                    nc.tensor.transpose(
                        pt[:, 128 * j : 128 * j + 128],
                        c3y[j][:, 128 * mo : 128 * mo + 128],
                        cw["identi"][:],
                    )
                c4y = pc4.tile([P, 512], IDT, tag="c4", name=f"c4y{b}{mo}")
                nc.scalar.activation(c4y[:], pt[:], Act.Copy)
                return c4y

            def back_b(items, split_last=False, mo_range=range(4)):
                """Inverse second half for one or more images, row-block
                interleaved: T2-y(m) -> iB (color coeffs folded into chroma
                weights; G precombined by PSUM accumulation) -> chroma evac
                -> fused upsample/recombine/clip -> store. T2-y is emitted
                one row-block ahead of its iB consumer so the PE never
                stalls on the c4 evac round-trip."""
                c4ys = {b: c4.pop("y0") if "y0" in c4 else t2y(b, c4, 0) for b, c4 in items}
                for mo in mo_range:
                    for b, c4 in items:
                        c4y = c4ys[b]
                        if mo < 3:
                            c4ys[b] = t2y(b, c4, mo + 1)
                            c4["y0"] = c4ys[b]
                        ypt = ps.tile([P, 512], DT, tag="ps", name=f"p_iby{b}{mo}")
                        nc.tensor.matmul(
                            ypt[:],
                            mk(cw["w_idct"][:]),
                            mk(c4y[:]),
                            start=True,
                            stop=True,
                        )
                        par = mo % 2
                        mps = slice(256 * (mo // 2), 256 * (mo // 2) + 256)
                        # R alone in one psum tile (1 matmul -> smallest evac
                        # -> clip-R starts earliest); G,B share the second
                        rpt = ps.tile([P, 256], DT, tag="ps", name=f"p_ibr{b}{mo}")
                        nc.tensor.matmul(
                            rpt[:],
                            mk(cw[f"w_ib{par}r"][:]),
                            mk(c4["cr"][:, mps]),
                            start=True,
                            stop=True,
                        )
                        gbp = ps.tile([P, 512], DT, tag="ps", name=f"p_ibgb{b}{mo}")
                        nc.tensor.matmul(
                            gbp[:, 0:256],
                            mk(cw[f"w_ib{par}g1"][:]),
                            mk(c4["cb"][:, mps]),
                            start=True,
                            stop=False,
                        )
                        nc.tensor.matmul(
                            gbp[:, 0:256],
                            mk(cw[f"w_ib{par}g2"][:]),
                            mk(c4["cr"][:, mps]),
                            start=False,
                            stop=True,
                        )
                        nc.tensor.matmul(
                            gbp[:, 256:512],
                            mk(cw[f"w_ib{par}b"][:]),
                            mk(c4["cb"][:, mps]),
                            start=True,
                            stop=True,
                        )
                        # evac chroma into one SBUF tile (R,G,B blocks)
                        cq3 = pcup.tile([P, 768], DT, tag="cup", name=f"cu{b}{mo}")
                        nc.scalar.activation(cq3[:, 0:256], rpt[:], Act.Copy)
                        nc.scalar.activation(cq3[:, 256:768], gbp[:], Act.Copy)

                        rows = slice(128 * mo, 128 * mo + 128)
                        # absorber: pull the PE-sem wait onto a 1x1 copy so
                        # the custom op carries at most one sync wait
                        ab = prgb.tile([1, 1], DT, tag="ab", name=f"ab{b}{mo}")
                        nc.vector.tensor_copy(ab[0:1, 0:1], ypt[0:1, 0:1])
                        # fused upsample/recombine/clip per channel:
                        # out[2j+k] = clip01(Y[2j+k] + cq3[256c + j])
                        rgb16 = prgb.tile(
                            [P, 1536],
                            mybir.dt.float16,
                            tag="rgb16",
                            bufs=4,
                            name=f"rgb16_{b}{mo}",
                        )
                        for c in range(3):
                            nc.vector._custom_dve(
                                CLIPSTT,
                                out=rgb16[:, 512 * c : 512 * c + 512],
                                in0=ypt[:],
                                in1=reap(
                                    cq3[:, 256 * c : 256 * c + 256],
                                    [[1, 256], [0, 2]],
                                ),
                                s0=1.0,
                            )
                        if split_last and mo >= 2:
                            # per-channel stores: the tail store begins after
                            # the first clip instead of after all three
                            for c in range(3):
                                nc.sync.dma_start(
                                    out=out[b][c, rows, :],
                                    in_=rgb16[:, 512 * c : 512 * c + 512],
                                )
                        else:
                            nc.sync.dma_start(
                                out=out[b][:, rows, :].rearrange("c h w -> h c w"),
                                in_=rgb16[:].rearrange("p (c f) -> p c f", c=3),
                            )

            # software-pipelined emission: A(b)+M(b) | bA(b-1)+bB(b-1) keeps
            # each engine's in-order queue interleaving two images; the final
            # two images' iB halves are row-block interleaved so the drain
            # keeps PE/ACT/DVE all fed until the last store
            pm = {}
            for b in range(NIMG):
                a = stage_a(b)
                dy = stage_m1(b, a)
                if b >= 1:
                    c3 = back_ia(b - 1, pm.pop(b - 1))
                dc = stage_m2(b, a)
                pm[b] = (dy, dc)
                if b >= 1:
                    st = back_t2(b - 1, c3)
                    st["y0"] = t2y(b - 1, st, 0)
                    back_b([(b - 1, st)])
            stl = back_t2(NIMG - 1, back_ia(NIMG - 1, pm.pop(NIMG - 1)))
            back_b([(NIMG - 1, stl)], split_last=True)

    nc.compile()
    return nc


# --------------------------------------------------------------------------
# entry point
# --------------------------------------------------------------------------
_last_results = None


_nc_cache = None


def kernel(image, y_table, c_table):
    global _last_results, _nc_cache
    from concourse import bass_utils

    image = np.ascontiguousarray(np.asarray(image), np.float32)
    packed, packedf, packedi = build_const_arrays(
        np.asarray(y_table), np.asarray(c_table)
    )

    # the program is table-independent (tables arrive as data through the
    # consts DRAM tensors), so repeat kernel() calls reuse the built module
    if _nc_cache is None:
        _nc_cache = build_program()
    nc = _nc_cache
    n_cores = 8
    per = image.shape[0] // n_cores
    in_maps = [
        {
            "img": np.ascontiguousarray(image[i * per : (i + 1) * per]),
            "consts": packed,
            "constsf": packedf,
            "constsi": packedi,
        }
        for i in range(n_cores)
    ]

    res = None
    last_exc = None
    for attempt in range(3):
        try:
            res = bass_utils.run_bass_kernel_spmd(
                nc,
                in_maps,
                core_ids=list(range(n_cores)),
                trace=os.environ.get("KERNEL_TRACE", "0") == "1",
            )
            break
        except Exception as e:  # transient NRT/device hiccups: retry
            last_exc = e
    if res is None:
        raise last_exc
    _last_results = res
    outs = [np.asarray(r["out"], np.float32) for r in res.results]
    return np.concatenate(outs, axis=0)


if __name__ == "__main__":
    rng = np.random.default_rng(0)
    img = rng.random((32, 3, 512, 512), np.float32)
    yt = np.ones((8, 8), np.float32)
    ct = np.ones((8, 8), np.float32)
    out = kernel(img, yt, ct)
    print("out", out.shape, out.dtype, float(out.min()), float(out.max()))

